# revision 17
# baseline (speedup 1.0000x reference)
"""Trainium2 Bass kernel for nn_DiffusionTestModel (GCNConv + dense head).

Math (reference):
    A[c, r]  = sym-normalized adjacency (incl. self loops)     [N, N]  (sparse, built dense on host)
    B        = A @ x                                           [N, N]
    aggT     = (B @ gcn_w.T).T = gcn_w @ B.T                   [N, N]
    H1T[k,c] = tanh(aggT[k,c] + gcn_b[k])                      [N, N]
    H2T[e,n] = tanh(sum_k wqT[k,e] H1T[k,n] + wq_b[e])         [E, N]  (E-sharded, never materialized)
    F[e]     = sum_n wf[n] H2T[e,n] + emb[e,:] @ wfe + wf_b    [E]

Device program per core j (SPMD over 8 cores):
    Phase A: BT_s[m, cl]   = sum_r x[r, m] * AT[r, j*SW+cl]    (GCN c-sharded: SW = N/8 cols per core)
    Phase B: H1T_s[k2, cl] = tanh(sum_m gwt[m, k2] * BT_s[m, cl] + gcn_b[k2])
             AllGather fires per completed k2 quarter, overlapping B's remaining compute
    Phase C: psum[e, n] = sum_k wqt[k, e] * H1T[k, n]  (wq/emb E-sharded; two n-shards
             cached at once to halve wqt re-reads); tanh(+wq_b) fused on evict;
             f[e] += sum_n wf[n] * H2T-tile  (DVE fused multiply-reduce, H2 stays in SBUF)

All pools are shared across phases (pool boundaries would serialize them);
PSUM runs as 4 double-buffered bank pairs. All matmuls run as float32r
(FP22 multiply, FP32 accumulate) at full PE rate.
"""

import os

import numpy as np

import concourse.bacc as bacc
import concourse.mybir as mybir
import concourse.tile as tile
from concourse.bass_utils import run_bass_kernel_spmd

F32 = mybir.dt.float32
F32R = mybir.dt.float32r
MM_DT = F32R  # matmul input dtype (float32r = FP22 multiply at full PE rate)
TANH = mybir.ActivationFunctionType.Tanh
MULT = mybir.AluOpType.mult
ADD = mybir.AluOpType.add

N = 4096          # nodes (= node feature dim)
E = 32768         # edges
EMB = 8
NCORES = 8
ES = E // NCORES  # edges per core
SW = N // NCORES  # GCN column-shard width per core
P = 128


def _mm_phase(nc, pools, kxn_groups, kxm_dram, MB, NB, n_rows, tag,
              evict_fn, cache_slots, psum_tag_fn, after_mb_fn=None):
    """out[m, c] = sum_k kxm[k, m] * kxn[k, c] over grouped kxn column blocks.

    kxn_groups: list of groups; each group is a list of "subs"; each sub is a
                list of KT [P, NB] DRAM APs (k-tiles of one NB-wide column
                block). All subs of a group are SBUF-cached together so the
                kxm stream is read once per group.
    kxm_dram:   [n_rows, M_total] DRAM AP, streamed as [P, MB] tiles
                (gpsimd DMA casts fp32 -> fp32r).
    cache_slots[s]: which shared cache-tag slot sub s uses.
    psum_tag_fn(i, s) -> shared psum tag name.
    evict_fn(nc, psum_ap, cb_global, mb, i): move the finished psum tile out.
    """
    cache_pool, stream_pool, psum_pool = pools
    KT = n_rows // P
    M_total = kxm_dram.shape[1]
    cb_base = 0
    for group in kxn_groups:
        cache = []
        for s in range(len(group)):
            subtiles = []
            for k in range(KT):
                t = cache_pool.tile([P, NB], MM_DT, name=f"{tag}_cache",
                                    tag=f"c{cache_slots[s]}_{k}")
                nc.gpsimd.dma_start(out=t, in_=group[s][k])
                subtiles.append(t)
            cache.append(subtiles)
        for mb in range(M_total // MB):
            psums = [[psum_pool.tile([P, NB], F32, name=f"{tag}_ps",
                                     tag=psum_tag_fn(i, s))
                      for s in range(len(group))] for i in range(MB // P)]
            for k in range(KT):
                st = stream_pool.tile([P, MB], MM_DT, name=f"{tag}_st", tag="stream")
                nc.gpsimd.dma_start(out=st, in_=kxm_dram[k * P:(k + 1) * P, mb * MB:(mb + 1) * MB])
                for i in range(MB // P):
                    for s in range(len(group)):
                        nc.tensor.matmul(
                            out=psums[i][s][:],
                            lhsT=st[:, i * P:(i + 1) * P],
                            rhs=cache[s][k][:],
                            start=(k == 0),
                            stop=(k == KT - 1),
                        )
            for i in range(MB // P):
                for s in range(len(group)):
                    evict_fn(nc, psums[i][s][:], cb_base + s, mb, i)
            if after_mb_fn is not None:
                after_mb_fn(nc, mb)
        cb_base += len(group)


def _col_block(dram_ap, n_rows, col0, NB):
    """k-tiles [P, NB] of dram_ap[:, col0:col0+NB]."""
    return [dram_ap[k * P:(k + 1) * P, col0:col0 + NB] for k in range(n_rows // P)]


def build_program(n=N, es=ES, ncores=NCORES, MB=512, MBC=256, NQ=4):
    """Build the per-core Bass program (identical across cores; data differs).

    MB: stream width for phases A/B; MBC: for phase C (smaller so C's psum
    block is 4 banks and double-buffers). NQ: AllGather split count.
    """
    nc = bacc.Bacc("TRN2", target_bir_lowering=False, debug=False)
    KT = n // P
    n_etiles = es // P
    sw = n // ncores
    quarter = n // NQ
    CGROUP = 2 if ncores % 2 == 0 else 1
    shared = "Shared" if (ncores > 4 and not os.environ.get("AG_LOCAL")) else "Local"

    x_d = nc.dram_tensor("x", [n, n], F32, kind="ExternalInput")
    at_d = nc.dram_tensor("at", [n, sw], F32, kind="ExternalInput")   # AT[:, my shard]
    gwt_d = nc.dram_tensor("gwt", [n, n], F32, kind="ExternalInput")
    gbt_d = nc.dram_tensor("gbt", [P, KT], F32, kind="ExternalInput")
    wqt_d = nc.dram_tensor("wqt", [n, es], F32, kind="ExternalInput")
    wqbt_d = nc.dram_tensor("wqbt", [P, n_etiles], F32, kind="ExternalInput")
    wfn_d = nc.dram_tensor("wfn", [P, n], F32, kind="ExternalInput")
    embr_d = nc.dram_tensor("embr", [P, n_etiles * EMB], F32, kind="ExternalInput")
    wfe_d = nc.dram_tensor("wfe", [P, EMB], F32, kind="ExternalInput")
    wfb_d = nc.dram_tensor("wfb", [P, 1], F32, kind="ExternalInput")
    out_d = nc.dram_tensor("out", [P, n_etiles], F32, kind="ExternalOutput")

    with tile.TileContext(nc) as tc:
        with tc.tile_pool(name="dram", bufs=1, space="DRAM") as dram, \
             tc.tile_pool(name="cachep", bufs=1) as cp, \
             tc.tile_pool(name="streamp", bufs=8) as sp, \
             tc.tile_pool(name="psump", bufs=2, space="PSUM") as pp, \
             tc.tile_pool(name="evictp", bufs=4) as ep, \
             tc.tile_pool(name="constp", bufs=1) as constp:

            bt_s = dram.tile([n, sw], F32, name="bt_s")
            ag_in = dram.tile([n, sw], F32, name="ag_in")
            h1t_q = [dram.tile([ncores * quarter, sw], F32, name=f"h1t_q{h}",
                               addr_space=shared) for h in range(NQ)]
            pools = (cp, sp, pp)

            # ---------- Phase A: BT_s = x.T @ AT_s ----------
            def evict_a(nc, psum, cb, mb, i):
                sb = ep.tile([P, sw], F32, name="a_ev", tag="a_ev")
                nc.vector.tensor_copy(out=sb[:], in_=psum)
                nc.sync.dma_start(
                    out=bt_s[mb * MB + i * P: mb * MB + (i + 1) * P, :],
                    in_=sb[:])

            _mm_phase(nc, pools, [[_col_block(at_d, n, 0, sw)]],
                      x_d[:, :], MB, sw, n, "a", evict_a,
                      cache_slots=[0], psum_tag_fn=lambda i, s: f"ps{i}")

            # ---------- Phase B: H1T_s = tanh(gwt.T @ BT_s + gcn_b) ----------
            gbt_sb = constp.tile([P, KT], F32, name="gbt_sb")
            nc.sync.dma_start(out=gbt_sb[:], in_=gbt_d[:, :])

            def evict_b(nc, psum, cb, mb, i):
                k2t = mb * (MB // P) + i
                sb = ep.tile([P, sw], F32, name="b_ev", tag="b_ev")
                nc.scalar.activation(sb[:], psum, TANH, bias=gbt_sb[:, k2t:k2t + 1])
                nc.sync.dma_start(
                    out=ag_in[mb * MB + i * P: mb * MB + (i + 1) * P, :],
                    in_=sb[:])

            def after_mb_b(nc, mb):
                done_rows = (mb + 1) * MB
                for h in range(NQ):
                    if done_rows - MB < (h + 1) * quarter <= done_rows:
                        nc.gpsimd.collective_compute(
                            "AllGather", mybir.AluOpType.bypass,
                            ins=[ag_in[h * quarter:(h + 1) * quarter, :]],
                            outs=[h1t_q[h][:]],
                            replica_groups=[list(range(ncores))],
                        )

            _mm_phase(nc, pools, [[_col_block(bt_s, n, 0, sw)]],
                      gwt_d[:, :], MB, sw, n, "b", evict_b,
                      cache_slots=[1], psum_tag_fn=lambda i, s: f"ps{i}",
                      after_mb_fn=after_mb_b)

            # ---------- Phase C: head (H2 stays on-chip) ----------
            wqbt_sb = constp.tile([P, n_etiles], F32, name="wqbt_sb")
            nc.sync.dma_start(out=wqbt_sb[:], in_=wqbt_d[:, :])
            wfn_sb = constp.tile([P, n], F32, name="wfn_sb")
            nc.sync.dma_start(out=wfn_sb[:], in_=wfn_d[:, :])
            f_acc = constp.tile([P, n_etiles], F32, name="f_acc")
            nc.vector.memset(f_acc[:], 0.0)

            def evict_c(nc, psum, cb, mb, i):
                et = mb * (MBC // P) + i
                n0 = cb * sw
                h2 = ep.tile([P, sw], F32, name="c_h2", tag="c_h2")
                nc.scalar.activation(h2[:], psum, TANH, bias=wqbt_sb[:, et:et + 1])
                scr = ep.tile([P, sw], F32, name="c_scr", tag="c_scr")
                fpart = ep.tile([P, 1], F32, name="c_fp", tag="c_fp")
                nc.vector.scalar_tensor_tensor(
                    out=scr[:], in0=h2[:], scalar=1.0,
                    in1=wfn_sb[:, n0:n0 + sw],
                    op0=MULT, op1=MULT, accum_out=fpart[:])
                nc.vector.tensor_add(f_acc[:, et:et + 1],
                                     f_acc[:, et:et + 1], fpart[:])

            # n-shard s is row-stacked in the gathered quarters: k-tile k of
            # shard s = h1t_q[k // (KT//NQ)] rows [s*quarter + (k % (KT//NQ))*P ...]
            def c_sub(s):
                tiles = []
                for k in range(KT):
                    h, r = divmod(k, KT // NQ)
                    tiles.append(h1t_q[h][s * quarter + r * P: s * quarter + (r + 1) * P, :])
                return tiles

            groups = [[c_sub(g * CGROUP + s) for s in range(CGROUP)]
                      for g in range(ncores // CGROUP)]
            _mm_phase(nc, pools, groups, wqt_d[:, :], MBC, sw, n, "c", evict_c,
                      cache_slots=[0, 1][:CGROUP],
                      psum_tag_fn=lambda i, s: f"ps{i * CGROUP + s}")

            # edge-embedding contribution + wf_b, then write out
            embr_sb = constp.tile([P, n_etiles * EMB], F32, name="embr_sb")
            nc.sync.dma_start(out=embr_sb[:], in_=embr_d[:, :])
            wfe_sb = constp.tile([P, EMB], F32, name="wfe_sb")
            nc.sync.dma_start(out=wfe_sb[:], in_=wfe_d[:, :])
            wfb_sb = constp.tile([P, 1], F32, name="wfb_sb")
            nc.sync.dma_start(out=wfb_sb[:], in_=wfb_d[:, :])
            scr9 = constp.tile([P, EMB], F32, name="c_scr9")
            fp9 = constp.tile([P, 1], F32, name="c_fp9")
            for t in range(n_etiles):
                nc.vector.scalar_tensor_tensor(
                    out=scr9[:], in0=embr_sb[:, t * EMB:(t + 1) * EMB],
                    scalar=1.0, in1=wfe_sb[:],
                    op0=MULT, op1=MULT, accum_out=fp9[:])
                nc.vector.tensor_add(f_acc[:, t:t + 1],
                                     f_acc[:, t:t + 1], fp9[:])
            out_sb = constp.tile([P, n_etiles], F32, name="out_sb")
            nc.vector.tensor_scalar_add(out_sb[:], f_acc[:], wfb_sb[:, 0:1])
            nc.sync.dma_start(out=out_d[:, :], in_=out_sb[:])

    nc.finalize()
    return nc


def host_inputs(x, edge_index, edge_weight, gcn_w, gcn_b, wq_w, wq_b, emb,
                wf_w, wf_b, n=N, e=E, ncores=NCORES):
    """Build the per-core input maps (host-side preprocessing)."""
    es = e // ncores
    sw = n // ncores
    n_etiles = es // P
    kt = n // P
    x = np.ascontiguousarray(np.asarray(x, dtype=np.float32))
    row = np.asarray(edge_index[0], dtype=np.int64)
    col = np.asarray(edge_index[1], dtype=np.int64)
    ew = np.asarray(edge_weight, dtype=np.float32)

    deg = np.zeros(n, dtype=np.float32)
    np.add.at(deg, col, ew)
    deg += 1.0  # self loops, weight 1
    dis = (1.0 / np.sqrt(deg)).astype(np.float32)

    at = np.zeros((n, n), dtype=np.float32)
    np.add.at(at, (row, col), dis[row] * ew * dis[col])
    idx = np.arange(n)
    at[idx, idx] += dis * dis

    gwt = np.ascontiguousarray(np.asarray(gcn_w, dtype=np.float32).T)
    gbt = np.ascontiguousarray(np.asarray(gcn_b, dtype=np.float32).reshape(kt, P).T)
    wfn = np.ascontiguousarray(np.broadcast_to(
        np.asarray(wf_w[0, :n], dtype=np.float32), (P, n)))
    wfe = np.ascontiguousarray(np.broadcast_to(
        np.asarray(wf_w[0, n:n + EMB], dtype=np.float32), (P, EMB)))
    wfb = np.full((P, 1), np.float32(np.asarray(wf_b).reshape(-1)[0]), dtype=np.float32)

    wq_w = np.asarray(wq_w, dtype=np.float32)
    wq_b = np.asarray(wq_b, dtype=np.float32)
    emb = np.asarray(emb, dtype=np.float32)

    in_maps = []
    for j in range(ncores):
        sl = slice(j * es, (j + 1) * es)
        wqt = np.ascontiguousarray(wq_w[sl, :].T)
        wqbt = np.ascontiguousarray(wq_b[sl].reshape(n_etiles, P).T)
        embr = np.ascontiguousarray(
            emb[sl].reshape(n_etiles, P, EMB).transpose(1, 0, 2).reshape(P, n_etiles * EMB))
        at_s = np.ascontiguousarray(at[:, j * sw:(j + 1) * sw])
        in_maps.append(dict(x=x, at=at_s, gwt=gwt, gbt=gbt, wqt=wqt, wqbt=wqbt,
                            wfn=wfn, embr=embr, wfe=wfe, wfb=wfb))
    return in_maps


_PROG = None


def kernel(**inputs):
    global _PROG
    in_maps = host_inputs(**inputs)
    if _PROG is None:
        _PROG = build_program()
    res = run_bass_kernel_spmd(_PROG, in_maps, core_ids=list(range(NCORES)))
    shards = [res.results[j]["out"].T.ravel() for j in range(NCORES)]
    return np.concatenate(shards).astype(np.float32)
